# revision 25
# baseline (speedup 1.0000x reference)
"""Causal self-attention on 8 Trainium2 NeuronCores.

Problem: B=4, T=2048, C=1024, H=16, DH=64.
  qkv = x @ w_qkv.T ; causal softmax attention per head ; y = attnout @ w_out.T

Sharding: 8 cores = 4 batches x 2 query-subsets. Each core computes the full
QKV projection for its batch (duplicated within the pair -> no collectives),
then attention for a load-balanced set of query rows (all 16 heads), then
the output projection for its own query rows against the full w_out.

Query balance under causality: global 512-row q-tiles are paired (i, 3-i):
  parity 0 -> q512 tiles [0, 3], parity 1 -> [1, 2] (20 key-tiles each).

Everything runs in "transposed space": Q^T/K^T are head-pair-stacked
[128=2x64 dh rows, T], scores are computed as S^T (keys on PSUM partitions),
and PV produces attnout^T directly. The V tile embeds all-ones 64-column
groups ([ones | A0 B0 ... A7 B7 | ones]); a two-group strided stationary
[vA|ones] / [ones|vB] makes each PV matmul also produce the softmax
denominator (replicated over the other 64 rows of its PSUM bank), so no
separate denominator matmuls are needed. Denominators are re-aligned to
their PV halves with partition-shifting SBUF->SBUF DMAs, inverted with one
fast approx reciprocal, and applied on the vector engine.

Phase overlap: attention for the first q-tile is interleaved with the
K projection (per head-pair) and the remaining V projection, so the
exp-bound attention work hides under projection matmuls; the scalar
engine's exp stream and the PE projection stream run concurrently.
"""

import threading

import numpy as np

B, T, C = 4, 2048, 1024
H = 16
DH = C // H
P = 128
TL = T // 2          # query rows per core
NPAIR = H // 2       # 8 head-pairs
NCT = C // P         # 8 c-tiles
QT_TILE = 512        # q columns per attention tile
NQT = TL // QT_TILE  # 2 local q-tiles

# local q512-tile -> global q512-tile, per parity (also the Q-proj map)
QMAP512 = [[0, 3], [1, 2]]

_cache = {}


def _build_program(parity: int):
    import concourse.mybir as mybir
    import concourse.tile as tile
    from concourse import bacc
    from concourse.masks import make_identity

    f32 = mybir.dt.float32
    bf16 = mybir.dt.bfloat16

    nc = bacc.Bacc("TRN2", target_bir_lowering=False, debug=False)
    x = nc.dram_tensor("x", [T, C], f32, kind="ExternalInput").ap()
    w_qkv = nc.dram_tensor("w_qkv", [3 * C, C], f32, kind="ExternalInput").ap()
    w_out = nc.dram_tensor("w_out", [C, C], f32, kind="ExternalInput").ap()
    y = nc.dram_tensor("y", [TL, C], f32, kind="ExternalOutput").ap()

    g512 = QMAP512[parity]
    G0, G1 = g512
    scale = 1.0 / float(np.sqrt(DH))

    with tile.TileContext(nc) as tc:
        with (
            tc.tile_pool(name="res", bufs=1) as res,
            tc.tile_pool(name="stage", bufs=2) as stage,
            tc.tile_pool(name="wtile", bufs=2) as wtile,
            tc.tile_pool(name="work", bufs=2) as work,
            tc.tile_pool(name="rdp", bufs=2) as rdp,
            tc.tile_pool(name="attn", bufs=2) as attnp,
            tc.tile_pool(name="yout", bufs=1) as yout,
            tc.tile_pool(name="xtp", bufs=1) as xtp,
            tc.tile_pool(name="pss", bufs=2, space="PSUM") as pss,
            tc.tile_pool(name="pso", bufs=4, space="PSUM") as pso,
        ):
            ident = res.tile([P, P], f32)
            make_identity(nc, ident)

            # ---- residents
            kT = res.tile([P, NPAIR, T], bf16)          # K^T   4 MB
            qT = res.tile([P, NPAIR, TL], bf16)         # Q^T   2 MB
            # V with an embedded all-ones block per pair: [vA|ones|vB]
            # (the matmul stationary must be one contiguous free dim)
            v = res.tile([P, T // P, NPAIR, 192], bf16)  # V+ones 6 MB
            for tt in range(T // P):
                nc.vector.memset(v[:, tt, :, 64:128], 1.0)
            wvT = res.tile([P, NCT, C], bf16)           # w_v^T 2 MB
            woT = res.tile([P, NCT, C], bf16)           # w_out^T 2 MB
            xT = xtp.tile([P, NCT, T], bf16)            # x^T   4 MB

            def load(src_ap):
                lf = stage.tile([P, C], f32, tag="ldf")
                nc.sync.dma_start(out=lf, in_=src_ap)
                return lf

            def transpose_block(lb, dst, dst_col):
                # f32 transposes, 4 per 1-bank PSUM tile; evacuation
                # copy casts f32->bf16
                for g in range(2):
                    pt = pso.tile([P, 512], f32, tag="po", name="pt")
                    ptr = pt.rearrange("p (a b) -> p a b", b=P)
                    for ci in range(4):
                        ct = 4 * g + ci
                        nc.tensor.transpose(
                            ptr[:, ci, :], lb[:, ct * P:(ct + 1) * P], ident)
                    nc.any.tensor_copy(
                        out=dst[:, 4 * g:4 * g + 4, dst_col:dst_col + P],
                        in_=ptr)

            # ================= phase A =================
            # x^T
            for tt in range(T // P):
                xb = load(x[tt * P:(tt + 1) * P, :])
                transpose_block(xb, xT, tt * P)

            # Q weights + Q projection (local q-tiles only)
            for fb in range(8):
                wb = load(w_qkv[fb * P:(fb + 1) * P, :])
                wqk = wtile.tile([P, NCT, P], bf16, tag="wqk")
                transpose_block(wb, wqk, 0)
                for u in range(NQT):
                    ps = pso.tile([P, 512], f32, tag="po", name="psq")
                    t0 = g512[u] * 512
                    for ct in range(NCT):
                        nc.tensor.matmul(
                            ps, wqk[:, ct, :], xT[:, ct, t0:t0 + 512],
                            start=(ct == 0), stop=(ct == NCT - 1),
                        )
                    nc.vector.tensor_copy(
                        out=qT[:, fb, u * 512:(u + 1) * 512], in_=ps)

            # V weights, then V projection for the tiles attention j=0 needs
            for fb in range(8):
                wb = load(w_qkv[(16 + fb) * P:(17 + fb) * P, :])
                transpose_block(wb, wvT, fb * P)

            def v_proj(tt):
                for fo in range(2):
                    ps = pso.tile([P, 512], f32, tag="po", name="psv")
                    for ct in range(NCT):
                        nc.tensor.matmul(
                            ps, xT[:, ct, tt * P:(tt + 1) * P],
                            wvT[:, ct, fo * 512:(fo + 1) * 512],
                            start=(ct == 0), stop=(ct == NCT - 1),
                        )
                    psr = ps.rearrange("p (a b) -> p a b", b=P)
                    nc.vector.tensor_copy(
                        out=v[:, tt, 4 * fo:4 * fo + 4, 0:64],
                        in_=psr[:, :, 0:64])
                    nc.vector.tensor_copy(
                        out=v[:, tt, 4 * fo:4 * fo + 4, 128:192],
                        in_=psr[:, :, 64:128])

            nv0 = 4 * (G0 + 1)          # V k-tiles needed by attention j=0
            for tt in range(nv0):
                v_proj(tt)

            # ================= attention machinery =================
            def attn_pair(p, j, G):
                """Emit scores/exp/PV for pair p of q-tile j; returns a
                closure that finishes the pair (reciprocal + normalize),
                to be emitted later so the partition-shift DMA latency
                hides under other work. Scores for k+1 are emitted before
                the PV of k so the PE never waits out a full exp."""
                nk = 4 * (G + 1)
                psA_t = pso.tile([P, QT_TILE], f32, tag="po", name="psA")
                psB_t = pso.tile([P, QT_TILE], f32, tag="po", name="psB")
                qA = qT[0:64, p, j * QT_TILE:(j + 1) * QT_TILE]
                qB = qT[64:128, p, j * QT_TILE:(j + 1) * QT_TILE]

                def scores(k):
                    s2 = pss.tile([P, 2, QT_TILE], f32, tag="s2")
                    ks = slice(k * P, (k + 1) * P)
                    nc.tensor.matmul(s2[:, 0, :], kT[0:64, p, ks], qA)
                    nc.tensor.matmul(s2[:, 1, :], kT[64:128, p, ks], qB)
                    return s2

                def exp_pv(k, s2):
                    p2 = work.tile([P, 2, QT_TILE], bf16, tag="p2")
                    nc.scalar.activation(
                        p2, s2, mybir.ActivationFunctionType.Exp,
                        scale=scale)
                    dj = k - 4 * G
                    if dj >= 0:  # diagonal tile: zero blocked cells
                        nc.gpsimd.affine_select(
                            out=p2, in_=p2,
                            compare_op=mybir.AluOpType.is_ge,
                            fill=0.0, base=-P * dj,
                            pattern=[[0, 2], [1, QT_TILE]],
                            channel_multiplier=-1)
                    st, sp_ = (k == 0), (k == nk - 1)
                    nc.tensor.matmul(           # [vA|ones]: PV_A ; D_A
                        psA_t, v[:, k, p, 0:128], p2[:, 0, :],
                        start=st, stop=sp_)
                    nc.tensor.matmul(           # [ones|vB]: D_B ; PV_B
                        psB_t, v[:, k, p, 64:192], p2[:, 1, :],
                        start=st, stop=sp_)

                s2_cur = scores(0)
                for k in range(nk):
                    s2_next = scores(k + 1) if k + 1 < nk else None
                    exp_pv(k, s2_cur)
                    s2_cur = s2_next
                # denominators -> SBUF (aligned), then partition-shift
                dS = rdp.tile([P, QT_TILE], f32, tag="dS")
                nc.vector.tensor_copy(out=dS[64:128], in_=psA_t[64:128])
                nc.vector.tensor_copy(out=dS[0:64], in_=psB_t[0:64])
                dD = rdp.tile([P, QT_TILE], f32, tag="dD")
                nc.sync.dma_start(out=dD[0:64], in_=dS[64:128])
                nc.sync.dma_start(out=dD[64:128], in_=dS[0:64])

                def finish(attnT):
                    rD = rdp.tile([P, QT_TILE], f32, tag="rD")
                    nc.vector.reciprocal_approx_fast(rD, dD)
                    nc.vector.tensor_mul(
                        out=attnT[0:64, p, :], in0=psA_t[0:64],
                        in1=rD[0:64])
                    nc.vector.tensor_mul(
                        out=attnT[64:128, p, :], in0=psB_t[64:128],
                        in1=rD[64:128])
                return finish

            def out_proj_item(attnT, j, sub, fo):
                qs = slice(sub * P, (sub + 1) * P)
                ps = pso.tile([P, 512], f32, tag="po", name="psy")
                for p in range(NPAIR):
                    nc.tensor.matmul(
                        ps, attnT[:, p, qs],
                        woT[:, p, fo * 512:(fo + 1) * 512],
                        start=(p == 0), stop=(p == NPAIR - 1),
                    )
                ysb = yout.tile([P, 512], f32, tag="ysb")
                nc.vector.tensor_copy(out=ysb, in_=ps)
                nc.sync.dma_start(
                    out=y[j * QT_TILE + sub * P:
                          j * QT_TILE + (sub + 1) * P,
                          fo * 512:(fo + 1) * 512],
                    in_=ysb)

            # ========== phase B: K proj + V proj + attention j=0 ==========
            # Per head-pair: K projection chunk 0 (all j=0 needs, since
            # G0 <= 1 chunk range is covered by u <= G0), then attention
            # j=0 for the pair, then the deferred K chunks and a share of
            # the remaining V projection. exp work starts ~90us earlier.
            def k_proj_u(wqk, p, u):
                ps = pso.tile([P, 512], f32, tag="po", name="psk")
                for ct in range(NCT):
                    nc.tensor.matmul(
                        ps, wqk[:, ct, :],
                        xT[:, ct, u * 512:(u + 1) * 512],
                        start=(ct == 0), stop=(ct == NCT - 1),
                    )
                nc.vector.tensor_copy(
                    out=kT[:, p, u * 512:(u + 1) * 512], in_=ps)

            vrest = list(range(nv0, T // P))
            attnT0 = attnp.tile([P, NPAIR, QT_TILE], bf16, tag="attnT")
            pend = None
            for p in range(NPAIR):
                wb = load(w_qkv[(8 + p) * P:(9 + p) * P, :])
                # finish the previous pair first: its PSUM banks free up
                # before this pair's projections need the ring, and the
                # shift-DMA latency hides under the transposes
                if pend is not None:
                    pend(attnT0)
                    pend = None
                wqk = wtile.tile([P, NCT, P], bf16, tag="wqk")
                transpose_block(wb, wqk, 0)
                for u in range(G0 + 1):
                    k_proj_u(wqk, p, u)
                pend = attn_pair(p, 0, G0)
                for u in range(G0 + 1, 4):
                    k_proj_u(wqk, p, u)
                # a share of remaining V tiles
                nshare = (len(vrest) + NPAIR - 1 - p) // (NPAIR - p) \
                    if p < NPAIR else 0
                for _ in range(min(nshare, len(vrest))):
                    v_proj(vrest.pop(0))
            assert not vrest
            pend(attnT0)

            # w_out transposes fill the PE while the last j=0 exps drain
            for fb in range(8):
                wb = load(w_out[fb * P:(fb + 1) * P, :])
                transpose_block(wb, woT, fb * P)

            # ========== phase C: attention j=1 + interleaved out-proj =====
            # ACT runs an exp backlog per pair, so one out-proj item per
            # pair start keeps the PE busy without starving the exps.
            attnT1 = attnp.tile([P, NPAIR, QT_TILE], bf16, tag="attnT")
            pend = None
            for p in range(NPAIR):
                if pend is not None:
                    pend(attnT1)
                out_proj_item(attnT0, 0, p // 2, p % 2)
                pend = attn_pair(p, 1, G1)
            pend(attnT1)
            for sub in range(4):
                for fo in range(2):
                    out_proj_item(attnT1, 1, sub, fo)

    nc.compile()
    return nc


def _get_program(parity: int):
    if parity not in _cache:
        _cache[parity] = _build_program(parity)
    return _cache[parity]


def _run_group(nc, in_maps, devices, out_holder, idx):
    """shard_map the program over `devices`, one in_map per device."""
    import jax
    from jax.sharding import Mesh, PartitionSpec
    from jax.experimental.shard_map import shard_map
    import concourse.mybir as mybir
    from concourse.bass2jax import (
        _bass_exec_p, install_neuronx_cc_hook, partition_id_tensor)

    install_neuronx_cc_hook()

    partition_name = (
        nc.partition_id_tensor.name if nc.partition_id_tensor else None)
    in_names, out_names, out_avals, zero_outs = [], [], [], []
    for alloc in nc.m.functions[0].allocations:
        if not isinstance(alloc, mybir.MemoryLocationSet):
            continue
        name = alloc.memorylocations[0].name
        if alloc.kind == "ExternalInput":
            if name != partition_name:
                in_names.append(name)
        elif alloc.kind == "ExternalOutput":
            out_names.append(name)
            shape = tuple(alloc.tensor_shape)
            dtype = mybir.dt.np(alloc.dtype)
            out_avals.append(jax.core.ShapedArray(shape, dtype))
            zero_outs.append(np.zeros(shape, dtype))
    n_params = len(in_names)
    n_outs = len(out_avals)
    all_names = in_names + out_names
    if partition_name is not None:
        all_names.append(partition_name)
    donate = tuple(range(n_params, n_params + n_outs))

    def _body(*args):
        operands = list(args)
        if partition_name is not None:
            operands.append(partition_id_tensor())
        outs = _bass_exec_p.bind(
            *operands,
            out_avals=tuple(out_avals),
            in_names=tuple(all_names),
            out_names=tuple(out_names),
            lowering_input_output_aliases=(),
            sim_require_finite=False,
            sim_require_nnan=False,
            nc=nc,
        )
        return tuple(outs)

    n = len(devices)
    mesh = Mesh(np.asarray(devices), ("core",))
    sharded = jax.jit(
        shard_map(
            _body, mesh=mesh,
            in_specs=(PartitionSpec("core"),) * (n_params + n_outs),
            out_specs=(PartitionSpec("core"),) * n_outs,
            check_rep=False,
        ),
        donate_argnums=donate, keep_unused=True,
    )
    concat_in = [
        np.concatenate([np.asarray(m[name]) for m in in_maps], axis=0)
        for name in in_names
    ]
    concat_zero = [
        np.zeros((n * z.shape[0], *z.shape[1:]), z.dtype) for z in zero_outs
    ]
    out_arrs = sharded(*concat_in, *concat_zero)
    out_holder[idx] = [
        {
            name: np.asarray(out_arrs[i]).reshape(n, *out_avals[i].shape)[c]
            for i, name in enumerate(out_names)
        }
        for c in range(n)
    ]


def kernel(x, attn_mask, w_qkv, w_out):
    """Full inputs in, full output out. attn_mask is all-ones (per the
    problem spec) so masking reduces to the causal structure."""
    import jax

    x = np.asarray(x, dtype=np.float32)
    w_qkv = np.asarray(w_qkv, dtype=np.float32)
    w_out = np.asarray(w_out, dtype=np.float32)

    nc_e = _get_program(0)
    nc_o = _get_program(1)

    devices = jax.devices()
    in_maps = [
        {"x": x[b], "w_qkv": w_qkv, "w_out": w_out} for b in range(B)
    ]

    results = [None, None]
    t_e = threading.Thread(
        target=_run_group, args=(nc_e, in_maps, devices[0:4], results, 0))
    t_o = threading.Thread(
        target=_run_group, args=(nc_o, in_maps, devices[4:8], results, 1))
    t_e.start(); t_o.start()
    t_e.join(); t_o.join()

    y = np.empty((B, T, C), dtype=np.float32)
    for parity, group in enumerate(results):
        for b in range(B):
            y_local = group[b]["y"]          # [TL, C] in local q order
            for j in range(NQT):
                G = QMAP512[parity][j]
                y[b, G * QT_TILE:(G + 1) * QT_TILE, :] = \
                    y_local[j * QT_TILE:(j + 1) * QT_TILE, :]
    return y


# revision 34
# speedup vs baseline: 1.2830x; 1.2830x over previous
"""Causal self-attention on 8 Trainium2 NeuronCores.

Problem: B=4, T=2048, C=1024, H=16, DH=64.
  qkv = x @ w_qkv.T ; causal softmax attention per head ; y = attnout @ w_out.T

Sharding: 8 cores = 4 batches x 2 query-subsets. Each core computes the full
QKV projection for its batch (duplicated within the pair -> no collectives),
then attention for a load-balanced set of query rows (all 16 heads), then
the output projection for its own query rows against the full w_out.

Query balance under causality: global 512-row q-tiles are paired (i, 3-i):
  parity 0 -> q512 tiles [0, 3], parity 1 -> [1, 2] (20 key-tiles each).

Everything runs in "transposed space": Q^T/K^T are head-pair-stacked
[128=2x64 dh rows, T], scores are computed as S^T (keys on PSUM partitions),
and PV produces attnout^T directly. The V tile embeds all-ones 64-column
groups ([ones | A0 B0 ... A7 B7 | ones]); a two-group strided stationary
[vA|ones] / [ones|vB] makes each PV matmul also produce the softmax
denominator (replicated over the other 64 rows of its PSUM bank), so no
separate denominator matmuls are needed. Denominators are re-aligned to
their PV halves with partition-shifting SBUF->SBUF DMAs, inverted with one
fast approx reciprocal, and applied on the vector engine.

Phase overlap: attention for the first q-tile is interleaved with the
K projection (per head-pair) and the remaining V projection, so the
exp-bound attention work hides under projection matmuls; the scalar
engine's exp stream and the PE projection stream run concurrently.
"""

import threading
from contextlib import ExitStack

import numpy as np

B, T, C = 4, 2048, 1024
H = 16
DH = C // H
P = 128
TL = T // 2          # query rows per core
NPAIR = H // 2       # 8 head-pairs
NCT = C // P         # 8 c-tiles
QT_TILE = 512        # q columns per attention tile
NQT = TL // QT_TILE  # 2 local q-tiles

# local q512-tile -> global q512-tile, per parity (also the Q-proj map)
QMAP512 = [[0, 3], [1, 2]]

_cache = {}


def _build_program(parity: int):
    import concourse.mybir as mybir
    import concourse.tile as tile
    from concourse import bacc
    from concourse.masks import make_identity

    f32 = mybir.dt.float32
    bf16 = mybir.dt.bfloat16

    nc = bacc.Bacc("TRN2", target_bir_lowering=False, debug=False)
    x = nc.dram_tensor("x", [T, C], f32, kind="ExternalInput").ap()
    w_qkv = nc.dram_tensor("w_qkv", [3 * C, C], f32, kind="ExternalInput").ap()
    w_out = nc.dram_tensor("w_out", [C, C], f32, kind="ExternalInput").ap()
    y = nc.dram_tensor("y", [TL, C], f32, kind="ExternalOutput").ap()

    g512 = QMAP512[parity]
    G0, G1 = g512
    scale = 1.0 / float(np.sqrt(DH))

    with tile.TileContext(nc) as tc:
        with (
            tc.tile_pool(name="res", bufs=1) as res,
            tc.tile_pool(name="stage", bufs=4) as stage,
            tc.tile_pool(name="wtile", bufs=2) as wtile,
            tc.tile_pool(name="work", bufs=2) as work,
            tc.tile_pool(name="rdp", bufs=2) as rdp,
            tc.tile_pool(name="attn", bufs=2) as attnp,
            tc.tile_pool(name="yout", bufs=1) as yout,
            tc.tile_pool(name="pss", bufs=2, space="PSUM") as pss,
            tc.tile_pool(name="pso", bufs=4, space="PSUM") as pso,
        ):
            ident = res.tile([P, P], f32)
            make_identity(nc, ident)

            # ---- residents
            kT = res.tile([P, NPAIR, T], bf16)          # K^T   4 MB
            qT = res.tile([P, NPAIR, TL], bf16)         # Q^T   2 MB
            # V with an embedded all-ones block per pair: [vA|ones|vB]
            # (the matmul stationary must be one contiguous free dim)
            v = res.tile([P, T // P, NPAIR, 192], bf16)  # V+ones 6 MB
            for tt in range(T // P):
                nc.vector.memset(v[:, tt, :, 64:128], 1.0)
            wvT = res.tile([P, NCT, C], bf16)           # w_v^T 2 MB
            # xT lives only through phase B; its 4 MB is reclaimed for
            # w_out^T before phase C
            xstack = ExitStack()
            xtp = xstack.enter_context(tc.tile_pool(name="xtp", bufs=1))
            xT = xtp.tile([P, NCT, T], bf16)            # x^T   4 MB

            def load(src_ap):
                lf = stage.tile([P, C], f32, tag="ldf")
                nc.sync.dma_start(out=lf, in_=src_ap)
                return lf

            def transpose_block(lb, dst, dst_col):
                # f32 transposes, 4 per 1-bank PSUM tile; evacuation
                # copy casts f32->bf16
                for g in range(2):
                    pt = pso.tile([P, 512], f32, tag="po", name="pt")
                    ptr = pt.rearrange("p (a b) -> p a b", b=P)
                    for ci in range(4):
                        ct = 4 * g + ci
                        nc.tensor.transpose(
                            ptr[:, ci, :], lb[:, ct * P:(ct + 1) * P], ident)
                    nc.any.tensor_copy(
                        out=dst[:, 4 * g:4 * g + 4, dst_col:dst_col + P],
                        in_=ptr)

            # ================= phase A =================
            # x^T
            for tt in range(T // P):
                xb = load(x[tt * P:(tt + 1) * P, :])
                transpose_block(xb, xT, tt * P)

            # Q weights + Q projection (local q-tiles only)
            for fb in range(8):
                wb = load(w_qkv[fb * P:(fb + 1) * P, :])
                wqk = wtile.tile([P, NCT, P], bf16, tag="wqk")
                transpose_block(wb, wqk, 0)
                for u in range(NQT):
                    ps = pso.tile([P, 512], f32, tag="po", name="psq")
                    t0 = g512[u] * 512
                    for ct in range(NCT):
                        nc.tensor.matmul(
                            ps, wqk[:, ct, :], xT[:, ct, t0:t0 + 512],
                            start=(ct == 0), stop=(ct == NCT - 1),
                        )
                    nc.vector.tensor_copy(
                        out=qT[:, fb, u * 512:(u + 1) * 512], in_=ps)

            # V weights, then V projection for the tiles attention j=0 needs
            for fb in range(8):
                wb = load(w_qkv[(16 + fb) * P:(17 + fb) * P, :])
                transpose_block(wb, wvT, fb * P)

            def v_proj(tt):
                for fo in range(2):
                    ps = pso.tile([P, 512], f32, tag="po", name="psv")
                    for ct in range(NCT):
                        nc.tensor.matmul(
                            ps, xT[:, ct, tt * P:(tt + 1) * P],
                            wvT[:, ct, fo * 512:(fo + 1) * 512],
                            start=(ct == 0), stop=(ct == NCT - 1),
                        )
                    psr = ps.rearrange("p (a b) -> p a b", b=P)
                    nc.vector.tensor_copy(
                        out=v[:, tt, 4 * fo:4 * fo + 4, 0:64],
                        in_=psr[:, :, 0:64])
                    nc.vector.tensor_copy(
                        out=v[:, tt, 4 * fo:4 * fo + 4, 128:192],
                        in_=psr[:, :, 64:128])

            nv0 = 4 * (G0 + 1)          # V k-tiles needed by attention j=0
            for tt in range(nv0):
                v_proj(tt)

            # ================= attention machinery =================
            def attn_pair(p, j, G):
                """Emit scores/exp/PV for pair p of q-tile j; returns a
                closure that finishes the pair (reciprocal + normalize),
                to be emitted later so the partition-shift DMA latency
                hides under other work. Scores for k+1 are emitted before
                the PV of k so the PE never waits out a full exp."""
                nk = 4 * (G + 1)
                psA_t = pso.tile([P, QT_TILE], f32, tag="po", name="psA")
                psB_t = pso.tile([P, QT_TILE], f32, tag="po", name="psB")
                qA = qT[0:64, p, j * QT_TILE:(j + 1) * QT_TILE]
                qB = qT[64:128, p, j * QT_TILE:(j + 1) * QT_TILE]

                def scores(k):
                    # diagonal tiles (dj>=1) have no live q-columns below
                    # 128*dj: restrict scores and exp to the live range
                    dj = k - 4 * G
                    c0 = P * dj if dj > 0 else 0
                    s2 = pss.tile([P, 2, QT_TILE], f32, tag="s2")
                    ks = slice(k * P, (k + 1) * P)
                    nc.tensor.matmul(s2[:, 0, c0:], kT[0:64, p, ks],
                                     qA[:, c0:])
                    nc.tensor.matmul(s2[:, 1, c0:], kT[64:128, p, ks],
                                     qB[:, c0:])
                    return s2

                def exp_pv(k, s2):
                    dj = k - 4 * G
                    c0 = P * dj if dj > 0 else 0
                    p2 = work.tile([P, 2, QT_TILE], bf16, tag="p2")
                    if c0:
                        nc.vector.memset(p2[:, :, 0:c0], 0.0)
                    nc.scalar.activation(
                        p2[:, :, c0:], s2[:, :, c0:],
                        mybir.ActivationFunctionType.Exp, scale=scale)
                    if dj >= 0:  # diagonal tile: zero blocked cells
                        nc.gpsimd.affine_select(
                            out=p2[:, :, c0:], in_=p2[:, :, c0:],
                            compare_op=mybir.AluOpType.is_ge,
                            fill=0.0, base=0,
                            pattern=[[0, 2], [1, QT_TILE - c0]],
                            channel_multiplier=-1)
                    st, sp_ = (k == 0), (k == nk - 1)
                    nc.tensor.matmul(           # [vA|ones]: PV_A ; D_A
                        psA_t, v[:, k, p, 0:128], p2[:, 0, :],
                        start=st, stop=sp_)
                    nc.tensor.matmul(           # [ones|vB]: D_B ; PV_B
                        psB_t, v[:, k, p, 64:192], p2[:, 1, :],
                        start=st, stop=sp_)

                s2_cur = scores(0)
                for k in range(nk):
                    s2_next = scores(k + 1) if k + 1 < nk else None
                    exp_pv(k, s2_cur)
                    s2_cur = s2_next
                # evacuate everything to SBUF at pair end so the PSUM
                # banks release immediately (they share a 4-deep ring
                # with the projection/out-proj tiles): denominators to
                # dS (then partition-shift DMAs), PV halves to pvS
                dS = rdp.tile([P, QT_TILE], f32, tag="dS", bufs=1)
                nc.vector.tensor_copy(out=dS[64:128], in_=psA_t[64:128])
                nc.vector.tensor_copy(out=dS[0:64], in_=psB_t[0:64])
                pvS = rdp.tile([P, QT_TILE], bf16, tag="pvS")
                nc.vector.tensor_copy(out=pvS[0:64], in_=psA_t[0:64])
                nc.vector.tensor_copy(out=pvS[64:128], in_=psB_t[64:128])
                dD = rdp.tile([P, QT_TILE], f32, tag="dD")
                nc.sync.dma_start(out=dD[0:64], in_=dS[64:128])
                nc.sync.dma_start(out=dD[64:128], in_=dS[0:64])

                def finish(attnT):
                    rD = rdp.tile([P, QT_TILE], f32, tag="rD")
                    nc.vector.reciprocal_approx_fast(rD, dD)
                    nc.vector.tensor_mul(
                        out=attnT[:, p, :], in0=pvS, in1=rD)
                return finish

            def out_proj_item(attnT, j, sub, fo):
                qs = slice(sub * P, (sub + 1) * P)
                ps = pso.tile([P, 512], f32, tag="po", name="psy")
                for p in range(NPAIR):
                    nc.tensor.matmul(
                        ps, attnT[:, p, qs],
                        woT[:, p, fo * 512:(fo + 1) * 512],
                        start=(p == 0), stop=(p == NPAIR - 1),
                    )
                ysb = yout.tile([P, 512], f32, tag="ysb")
                nc.vector.tensor_copy(out=ysb, in_=ps)
                nc.sync.dma_start(
                    out=y[j * QT_TILE + sub * P:
                          j * QT_TILE + (sub + 1) * P,
                          fo * 512:(fo + 1) * 512],
                    in_=ysb)

            # ========== phase B: K proj + V proj + attention j=0 ==========
            # Per head-pair: K projection chunk 0 (all j=0 needs, since
            # G0 <= 1 chunk range is covered by u <= G0), then attention
            # j=0 for the pair, then the deferred K chunks and a share of
            # the remaining V projection. exp work starts ~90us earlier.
            def k_proj_u(wqk, p, u):
                ps = pso.tile([P, 512], f32, tag="po", name="psk")
                for ct in range(NCT):
                    nc.tensor.matmul(
                        ps, wqk[:, ct, :],
                        xT[:, ct, u * 512:(u + 1) * 512],
                        start=(ct == 0), stop=(ct == NCT - 1),
                    )
                nc.vector.tensor_copy(
                    out=kT[:, p, u * 512:(u + 1) * 512], in_=ps)

            vrest = list(range(nv0, T // P))
            attnT0 = attnp.tile([P, NPAIR, QT_TILE], bf16, tag="attnT")
            pend = None
            for p in range(NPAIR):
                wb = load(w_qkv[(8 + p) * P:(9 + p) * P, :])
                # finish the previous pair first: its PSUM banks free up
                # before this pair's projections need the ring, and the
                # shift-DMA latency hides under the transposes
                if pend is not None:
                    pend(attnT0)
                    pend = None
                wqk = wtile.tile([P, NCT, P], bf16, tag="wqk")
                transpose_block(wb, wqk, 0)
                for u in range(G0 + 1):
                    k_proj_u(wqk, p, u)
                pend = attn_pair(p, 0, G0)
                for u in range(G0 + 1, 4):
                    k_proj_u(wqk, p, u)
                # a share of remaining V tiles
                nshare = (len(vrest) + NPAIR - 1 - p) // (NPAIR - p) \
                    if p < NPAIR else 0
                for _ in range(min(nshare, len(vrest))):
                    v_proj(vrest.pop(0))
            assert not vrest
            pend(attnT0)

            # reclaim xT's SBUF for w_out^T, whose transposes fill the
            # PE while the last j=0 exps drain
            xstack.close()
            wop = xstack.enter_context(tc.tile_pool(name="wop", bufs=1))
            woT = wop.tile([P, NCT, C], bf16)           # w_out^T 2 MB
            for fb in range(8):
                wb = load(w_out[fb * P:(fb + 1) * P, :])
                transpose_block(wb, woT, fb * P)

            # ========== phase C: attention j=1 + interleaved out-proj =====
            # ACT runs an exp backlog per pair, so one out-proj item per
            # pair start keeps the PE busy without starving the exps.
            attnT1 = attnp.tile([P, NPAIR, QT_TILE], bf16, tag="attnT")
            pend = None
            for p in range(NPAIR):
                if pend is not None:
                    pend(attnT1)
                out_proj_item(attnT0, 0, p // 2, p % 2)
                pend = attn_pair(p, 1, G1)
            pend(attnT1)
            for sub in range(4):
                for fo in range(2):
                    out_proj_item(attnT1, 1, sub, fo)
            xstack.close()

    nc.compile()
    return nc


def _get_program(parity: int):
    if parity not in _cache:
        _cache[parity] = _build_program(parity)
    return _cache[parity]


def _run_group(nc, in_maps, devices, out_holder, idx):
    """shard_map the program over `devices`, one in_map per device."""
    import jax
    from jax.sharding import Mesh, PartitionSpec
    from jax.experimental.shard_map import shard_map
    import concourse.mybir as mybir
    from concourse.bass2jax import (
        _bass_exec_p, install_neuronx_cc_hook, partition_id_tensor)

    install_neuronx_cc_hook()

    partition_name = (
        nc.partition_id_tensor.name if nc.partition_id_tensor else None)
    in_names, out_names, out_avals, zero_outs = [], [], [], []
    for alloc in nc.m.functions[0].allocations:
        if not isinstance(alloc, mybir.MemoryLocationSet):
            continue
        name = alloc.memorylocations[0].name
        if alloc.kind == "ExternalInput":
            if name != partition_name:
                in_names.append(name)
        elif alloc.kind == "ExternalOutput":
            out_names.append(name)
            shape = tuple(alloc.tensor_shape)
            dtype = mybir.dt.np(alloc.dtype)
            out_avals.append(jax.core.ShapedArray(shape, dtype))
            zero_outs.append(np.zeros(shape, dtype))
    n_params = len(in_names)
    n_outs = len(out_avals)
    all_names = in_names + out_names
    if partition_name is not None:
        all_names.append(partition_name)
    donate = tuple(range(n_params, n_params + n_outs))

    def _body(*args):
        operands = list(args)
        if partition_name is not None:
            operands.append(partition_id_tensor())
        outs = _bass_exec_p.bind(
            *operands,
            out_avals=tuple(out_avals),
            in_names=tuple(all_names),
            out_names=tuple(out_names),
            lowering_input_output_aliases=(),
            sim_require_finite=False,
            sim_require_nnan=False,
            nc=nc,
        )
        return tuple(outs)

    n = len(devices)
    mesh = Mesh(np.asarray(devices), ("core",))
    sharded = jax.jit(
        shard_map(
            _body, mesh=mesh,
            in_specs=(PartitionSpec("core"),) * (n_params + n_outs),
            out_specs=(PartitionSpec("core"),) * n_outs,
            check_rep=False,
        ),
        donate_argnums=donate, keep_unused=True,
    )
    concat_in = [
        np.concatenate([np.asarray(m[name]) for m in in_maps], axis=0)
        for name in in_names
    ]
    concat_zero = [
        np.zeros((n * z.shape[0], *z.shape[1:]), z.dtype) for z in zero_outs
    ]
    out_arrs = sharded(*concat_in, *concat_zero)
    out_holder[idx] = [
        {
            name: np.asarray(out_arrs[i]).reshape(n, *out_avals[i].shape)[c]
            for i, name in enumerate(out_names)
        }
        for c in range(n)
    ]


def kernel(x, attn_mask, w_qkv, w_out):
    """Full inputs in, full output out. attn_mask is all-ones (per the
    problem spec) so masking reduces to the causal structure."""
    import jax

    x = np.asarray(x, dtype=np.float32)
    w_qkv = np.asarray(w_qkv, dtype=np.float32)
    w_out = np.asarray(w_out, dtype=np.float32)

    nc_e = _get_program(0)
    nc_o = _get_program(1)

    devices = jax.devices()
    in_maps = [
        {"x": x[b], "w_qkv": w_qkv, "w_out": w_out} for b in range(B)
    ]

    results = [None, None]
    t_e = threading.Thread(
        target=_run_group, args=(nc_e, in_maps, devices[0:4], results, 0))
    t_o = threading.Thread(
        target=_run_group, args=(nc_o, in_maps, devices[4:8], results, 1))
    t_e.start(); t_o.start()
    t_e.join(); t_o.join()

    y = np.empty((B, T, C), dtype=np.float32)
    for parity, group in enumerate(results):
        for b in range(B):
            y_local = group[b]["y"]          # [TL, C] in local q order
            for j in range(NQT):
                G = QMAP512[parity][j]
                y[b, G * QT_TILE:(G + 1) * QT_TILE, :] = \
                    y_local[j * QT_TILE:(j + 1) * QT_TILE, :]
    return y
